# revision 18
# baseline (speedup 1.0000x reference)
"""BlockStackingSGN kernel for 8 Trainium2 NeuronCores.

Data-parallel over batch B=4096 (512 rows/core; batch in the free dim,
hidden on partitions). Key optimizations over a bf16 tiling:

- fp8e4m3 DoubleRow matmuls for every 256-deep contraction: one PE
  instruction contracts both 128-row k-tiles in the cycles of one,
  halving PE time.
- The linear object-encoder output layer (no relu) is folded on the host
  into its four downstream consumers (AonB-left/right, clear, ontable
  first layers), deleting that layer's matmuls and evacuations.
- Power-of-2 scaling (weights x16) keeps fp8 weights out of the
  subnormal range; scales flow through relu/add transparently and are
  absorbed for free by activation-engine scale or a tensor_scalar
  multiply, so every PSUM evacuation is a single instruction.
- Early phases run two 256-wide layers per 4-bank PSUM tile so one
  evacuation instruction drains four matmul accumulations (GpSimd
  cannot read PSUM, so evacuations are split across Scalar+Vector only;
  GpSimd handles the SBUF-side pair adds and relu casts).
- All 80 output heads (AonB pairs / clear / ontable) accumulate into one
  PSUM bank via one-hot fp8 stationaries sliced from a sliding window;
  a single batched Sigmoid finishes the kernel.
"""

import sys

import numpy as np

sys.path.insert(0, "/opt/trn_rl_repo")

import concourse.bacc as bacc
import concourse.mybir as mybir
import concourse.tile as tile
from concourse.bass_utils import run_bass_kernel_spmd

dt = mybir.dt
AF = mybir.ActivationFunctionType
ALU = mybir.AluOpType
PM = mybir.MatmulPerfMode

N = 8
H = 256
B = 4096
IN = 3 * N
NCORES = 8
BC = B // NCORES          # 512 batch rows per core
W = BC
R = N * (N + 2)           # 80 output rows
S = 16.0                  # weight scale 2^4

F32 = dt.float32
BF16 = dt.bfloat16
FP8 = dt.float8e4

_CACHE = {}


def _wb_layout():
    """fp8 weight tile entries of [128, 2, 256] (512 cols each), ordered by
    first use (doubles as DMA arrival order)."""
    keys = []
    for n in range(N):
        keys.append(("oW1", n))
    for n in range(N):
        keys.append(("Wl", n))
        keys.append(("Wr", n))
    for n in range(N):
        keys.append(("Wc", n))
        keys.append(("Wt", n))
    keys += [("cW1",), ("tW1",), ("w2c",), ("w2t",), ("ident",), ("aW1",),
             ("w2a",)]
    return {k: i for i, k in enumerate(keys)}, len(keys)


WB_ENT, WB_N = _wb_layout()
WB_COLS = WB_N * 512


def _bias_layout():
    keys = []
    for n in range(N):
        for nm in ("b0", "b1", "bl", "br", "bc", "bt"):
            for m in range(2):
                keys.append((nm, n, m))
    for nm in ("cb1", "tb1", "ab1", "zz"):
        for m in range(2):
            keys.append((nm, m))
    keys.append(("finb",))
    return {k: i for i, k in enumerate(keys)}, len(keys)


BIAS_OFF, BIAS_COLS = _bias_layout()

N_DMA_CHUNKS = 8


def _build(zero_bias):
    nc = bacc.Bacc("TRN2", target_bir_lowering=False, debug=False, num_devices=NCORES)

    d_x = nc.dram_tensor("x", [IN, BC], BF16, kind="ExternalInput")
    d_ow0 = nc.dram_tensor("ow0", [IN, N * H], BF16, kind="ExternalInput")
    d_wb = nc.dram_tensor("wb", [128, WB_COLS], FP8, kind="ExternalInput")
    d_bias = nc.dram_tensor("bias", [128, BIAS_COLS], F32, kind="ExternalInput")
    d_out = nc.dram_tensor("outT", [R, BC], F32, kind="ExternalOutput")

    K2 = 2 * W   # 1024: one 256-wide activation (2 k-tiles x 512 batch)

    with tile.TileContext(nc) as tc:
        with (
            tc.tile_pool(name="w", bufs=1) as wp,
            tc.tile_pool(name="act", bufs=1) as acp,
            tc.tile_pool(name="wk", bufs=8) as wk,
            tc.tile_pool(name="ph2", bufs=10) as php,
        ):
            xw = wp.tile([IN, BC], BF16, tag="xw")
            nc.sync.dma_start(xw[:], d_x[:])
            xT = xw[:]
            ow0 = wp.tile([IN, N * H], BF16, tag="ow0")
            nc.scalar.dma_start(ow0[:, :2 * H], d_ow0[:, :2 * H])
            nc.gpsimd.dma_start(ow0[:, 2 * H:], d_ow0[:, 2 * H:])
            bias = wp.tile([128, BIAS_COLS], F32, tag="bias")
            nc.gpsimd.dma_start(bias[:], d_bias[:])

            wb = wp.tile([128, WB_N, 2, 256], FP8, tag="wb")
            chunk = (WB_N + N_DMA_CHUNKS - 1) // N_DMA_CHUNKS
            for c in range(N_DMA_CHUNKS):
                eng = nc.gpsimd if c % 2 == 0 else nc.sync
                lo, hi = c * chunk, min((c + 1) * chunk, WB_N)
                if lo < hi:
                    eng.dma_start(wb[:, lo:hi], d_wb[:, lo * 512:hi * 512])

            def wsl(key, m):
                return wb[:, WB_ENT[key], :, m * 128:(m + 1) * 128]

            def w2sl(key, r):
                return wb[:, WB_ENT[key], :, 128 - r:256 - r]

            def bcol(key):
                return bias[:, BIAS_OFF[key]:BIAS_OFF[key] + 1]

            def engine(e):
                return {"A": nc.scalar, "D": nc.vector, "P": nc.gpsimd}[e]

            def asdr(ap2d):
                """view a [128, 1024] activation slice as DR rhs [128, 2, 512]"""
                return ap2d.rearrange("p (k w) -> p k w", k=2)

            def evac(e, out2, ps2, bkeys, relu, scale):
                """out = func(scale * psum + scale*bias). One instruction when
                biases are zero, else one per [128, W] column block."""
                if zero_bias:
                    if e == "A":
                        func = AF.Relu if relu else AF.Identity
                        nc.scalar.activation(out2, ps2, func, scale=scale)
                    elif relu:
                        # (mult scale, max 0): measured faster than plain max
                        engine(e).tensor_scalar(out2, ps2, scale, 0.0,
                                                ALU.mult, ALU.max)
                    else:
                        engine(e).tensor_scalar(out2, ps2, scale, None, ALU.mult)
                else:
                    nsub = out2.shape[-1] // W if len(out2.shape) == 2 else 2
                    for m in range(nsub):
                        o = out2[:, m * W:(m + 1) * W]
                        p = ps2[:, m * W:(m + 1) * W]
                        b = bcol(bkeys[m])
                        if e == "A" or (relu and scale != 1.0):
                            func = AF.Relu if relu else AF.Identity
                            nc.scalar.activation(o, p, func, bias=b, scale=scale)
                        elif relu:
                            engine(e).tensor_scalar(o, p, b, 0.0, ALU.add, ALU.max)
                        else:
                            engine(e).tensor_scalar(o, p, b, scale,
                                                    ALU.add, ALU.mult)

            cts = {}

            def pick(seq, key):
                c = cts.setdefault(key, [0])
                e = seq[c[0] % len(seq)]
                c[0] += 1
                return e

            SEQ_ERA = "ADAD"      # era-A 4-bank evacs
            SEQ_Y1 = "AD"
            SEQ_YE = "DADA"       # pair y evacs

            # ================= era A: 4-bank psum tiles =================
            h0 = acp.tile([128, N * K2], FP8, tag="h0")
            h1 = acp.tile([128, N * K2], FP8, tag="h1")
            alr = acp.tile([128, 2 * N * K2], FP8, tag="alr")

            def blk(t, n, q=1):
                return t[:, n * K2:(n + q) * K2]

            import bass_rust as _br

            def pair_rhs(i, j, m):
                """[128, 2, 512] AP: dim1 hops from al_i[m] to ar_j[m]."""
                sl = alr[:, 2 * i * K2 + m * W: 2 * i * K2 + (m + 1) * W].copy()
                step = (2 * j + 1 - 2 * i) * K2
                sl.ap = _br.VecI64Pair([[2 * N * K2, 128], [step, 2], [1, W]])
                return sl

            with tc.tile_pool(name="pa", bufs=2, space="PSUM") as pa:
                # L0 (bf16, contraction 24): two blocks per psum tile
                for n in range(0, N, 2):
                    pst = pa.tile([128, 2 * K2], F32, tag="pa", name=f"psA{n}")
                    for q in range(2):
                        for m in range(2):
                            o = (n + q) * H + m * 128
                            nc.tensor.matmul(
                                pst[:, (2 * q + m) * W:(2 * q + m + 1) * W],
                                ow0[:, o:o + 128], xT, start=True, stop=True,
                                skip_group_check=True)
                    evac(pick(SEQ_ERA, "h0"), blk(h0, n, 2), pst[:],
                         [("b0", n + q, m) for q in range(2) for m in range(2)],
                         True, 1.0)
                # L1 (DR)
                for n in range(0, N, 2):
                    pst = pa.tile([128, 2 * K2], F32, tag="pa", name=f"psB{n}")
                    for q in range(2):
                        for m in range(2):
                            nc.tensor.matmul(
                                pst[:, (2 * q + m) * W:(2 * q + m + 1) * W],
                                wsl(("oW1", n + q), m), asdr(blk(h0, n + q)),
                                start=True, stop=True, perf_mode=PM.DoubleRow,
                                skip_group_check=True)
                    evac(pick(SEQ_ERA, "h1"), blk(h1, n, 2), pst[:],
                         [("b1", n + q, m) for q in range(2) for m in range(2)],
                         True, 1.0)
                # al / ar (DR, enc folded; fp8 out at 2^4). One block per
                # 4-bank psum tile: [al_n m0 m1 | ar_n m0 m1] so each block's
                # pair inputs land together (pairs are shell-ordered below)
                for n in range(N):
                    pst = pa.tile([128, 2 * K2], F32, tag="pa", name=f"psC{n}")
                    for q, key in enumerate(("Wl", "Wr")):
                        for m in range(2):
                            nc.tensor.matmul(
                                pst[:, (2 * q + m) * W:(2 * q + m + 1) * W],
                                wsl((key, n), m), asdr(blk(h1, n)),
                                start=True, stop=True, perf_mode=PM.DoubleRow,
                                skip_group_check=True)
                    evac(pick(SEQ_ERA, "al"), alr[:, 2 * n * K2:(2 * n + 2) * K2],
                         pst[:],
                         [(bk, n, m) for bk in ("bl", "br") for m in range(2)],
                         False, 1.0 / (S * S))

            # ================= era B: pair loop + preds =================
            with (
                tc.tile_pool(name="py", bufs=3, space="PSUM") as py,
                tc.tile_pool(name="pf", bufs=1, space="PSUM") as pf,
            ):
                pp = py
                fin = pf.tile([128, BC], F32, tag="fin")
                n_fin = N * N + 2 * N
                fin_ct = [0]

                def fin_mm(w2key, r, rhs2):
                    first = fin_ct[0] == 0
                    fin_ct[0] += 1
                    last = fin_ct[0] == n_fin
                    nc.tensor.matmul(fin[:], w2sl(w2key, r), asdr(rhs2),
                                     start=first, stop=last,
                                     perf_mode=PM.DoubleRow)

                def dr2(ps2, key, rhs2):
                    for m in range(2):
                        nc.tensor.matmul(ps2[:, m * W:(m + 1) * W], wsl(key, m),
                                         asdr(rhs2), start=True, stop=True,
                                         perf_mode=PM.DoubleRow,
                                         skip_group_check=True)

                def pred_stages(n, w0k, w1k, b0k, b1k, w2k, r):
                    """three independently schedulable stages of one predicate"""
                    st = {}

                    def s1():
                        y0 = wk.tile([128, K2], FP8, tag="y0",
                                     name=f"y0_{n}_{w0k}")
                        pst = pp.tile([128, K2], F32, tag="py",
                                      name=f"psY0{n}{w0k}")
                        dr2(pst, (w0k, n), blk(h1, n))
                        evac("A", y0[:], pst[:],
                             [(b0k, n, m) for m in range(2)], True, 1.0 / (S * S))
                        st["y0"] = y0

                    def s2():
                        y1 = wk.tile([128, K2], FP8, tag="y1",
                                     name=f"y1_{n}_{w0k}")
                        pst2 = pp.tile([128, K2], F32, tag="py",
                                       name=f"psY1{n}{w0k}")
                        dr2(pst2, w1k, st["y0"][:])
                        evac(pick(SEQ_Y1, "y1"), y1[:], pst2[:],
                             [(b1k, m) for m in range(2)], True, 1.0)
                        st["y1"] = y1

                    def s3():
                        fin_mm(w2k, r, st["y1"][:])

                    return [s1, s2, s3]

                allst = []
                for n in range(N):
                    allst.append(pred_stages(n, "Wc", ("cW1",), "bc", "cb1",
                                             ("w2c",), n * 10 + 8))
                    allst.append(pred_stages(n, "Wt", ("tW1",), "bt", "tb1",
                                             ("w2t",), n * 10 + 9))
                # interleave stages of pred pairs: A1 B1 A2 B2 A3 B3
                stageq = []
                for pn in range(0, 16, 2):
                    A, Bq = allst[pn], allst[pn + 1]
                    stageq += [A[0], Bq[0], A[1], Bq[1], A[2], Bq[2]]

                # modulo-scheduled pair loop: ident leads, aW1 lags 1 pair,
                # fin lags 4 pairs; pred stages fill the gaps
                PAIRS = sorted(((i, j) for i in range(N)
                                for j in range(N)),
                               key=lambda p: (max(p), p))
                PHQ, YQ = {}, {}

                def emit_ident(k):
                    i, j = PAIRS[k]
                    pst1 = py.tile([128, K2], F32, tag="py", name=f"psH{i}{j}")
                    for m in range(2):
                        nc.tensor.matmul(pst1[:, m * W:(m + 1) * W],
                                         wsl(("ident",), 0), pair_rhs(i, j, m),
                                         start=True, stop=True,
                                         perf_mode=PM.DoubleRow,
                                         skip_group_check=True)
                    ph = php.tile([128, K2], FP8, tag="ph", name=f"ph{i}{j}")
                    evac(pick("ADAD", "ph"), ph[:], pst1[:],
                         [("zz", m) for m in range(2)], True, 1.0)
                    PHQ[k] = ph

                def emit_aw1(k):
                    i, j = PAIRS[k]
                    ph = PHQ.pop(k)
                    pst = py.tile([128, K2], F32, tag="py", name=f"psP{i}{j}")
                    for m in range(2):
                        nc.tensor.matmul(
                            pst[:, m * W:(m + 1) * W], wsl(("aW1",), m),
                            asdr(ph[:]), start=True, stop=True,
                            perf_mode=PM.DoubleRow, skip_group_check=True)
                    y = wk.tile([128, K2], FP8, tag="y", name=f"y_{i}{j}")
                    evac(pick(SEQ_YE, "ye"), y[:], pst[:],
                         [("ab1", m) for m in range(2)], True, 1.0)
                    YQ[k] = y

                def emit_fin(k):
                    i, j = PAIRS[k]
                    y = YQ.pop(k)
                    fin_mm(("w2a",), i * 10 + j, y[:])

                NP = len(PAIRS)
                for k in range(NP + 4):
                    if k < NP:
                        emit_ident(k)
                    if k % 4 != 0 and stageq:
                        stageq.pop(0)()
                    if 1 <= k <= NP:
                        emit_aw1(k - 1)
                    if k >= 4:
                        emit_fin(k - 4)
                while stageq:
                    stageq.pop(0)()
                assert fin_ct[0] == n_fin

                # batched sigmoid + store
                outT = wk.tile([128, BC], F32, tag="outT")
                nc.scalar.activation(outT[:], fin[:], AF.Sigmoid,
                                     bias=bcol(("finb",)), scale=1.0 / (S ** 3))
                nc.sync.dma_start(d_out[:], outT[:R, :])

    nc.compile()
    return nc


def _prep_inputs(inputs):
    import ml_dtypes

    bf = ml_dtypes.bfloat16
    f8 = ml_dtypes.float8_e4m3fn
    f32a = lambda a: np.asarray(a, dtype=np.float32)

    wbv = np.zeros((128, WB_N, 2, 256), f8)

    def put(key, Wmat):  # Wmat: [256, 256] fp32, already scaled
        e = WB_ENT[key]
        for k in range(2):
            wbv[:, e, k, :] = Wmat[k * 128:(k + 1) * 128].astype(f8)

    oW1 = f32a(inputs["o_W1"])
    oW2 = f32a(inputs["o_W2"])
    aW0 = f32a(inputs["a_W0"])
    cW0 = f32a(inputs["c_W0"])
    tW0 = f32a(inputs["t_W0"])
    for n in range(N):
        put(("oW1", n), S * oW1[n])
        put(("Wl", n), S * (oW2[n] @ aW0[:H]))
        put(("Wr", n), S * (oW2[n] @ aW0[H:]))
        put(("Wc", n), S * (oW2[n] @ cW0))
        put(("Wt", n), S * (oW2[n] @ tW0))
    eye = np.zeros((2 * 128, 256), np.float32)
    eye[:128, :128] = np.eye(128)
    eye[128:, :128] = np.eye(128)
    put(("ident",), eye)
    put(("cW1",), S * f32a(inputs["c_W1"]))
    put(("tW1",), S * f32a(inputs["t_W1"]))
    put(("aW1",), S * f32a(inputs["a_W1"]))
    for key, src in ((("w2c",), "c_W2"), (("w2t",), "t_W2"), (("w2a",), "a_W2")):
        w2 = S * f32a(inputs[src])[:, 0]
        e = WB_ENT[key]
        for k in range(2):
            wbv[:, e, k, 128] = w2[k * 128:(k + 1) * 128].astype(f8)

    biasv = np.zeros((128, BIAS_COLS), np.float32)

    def putb(key, vec):
        biasv[:, BIAS_OFF[key]] = vec

    ob2 = f32a(inputs["o_b2"])
    blv = ob2 @ aW0[:H] + f32a(inputs["a_b0"])[None, :]
    brv = ob2 @ aW0[H:]
    bcv = ob2 @ cW0 + f32a(inputs["c_b0"])[None, :]
    btv = ob2 @ tW0 + f32a(inputs["t_b0"])[None, :]
    for n in range(N):
        for m in range(2):
            sl = slice(m * 128, (m + 1) * 128)
            putb(("b0", n, m), S * f32a(inputs["o_b0"])[n][sl])
            putb(("b1", n, m), S * S * f32a(inputs["o_b1"])[n][sl])
            putb(("bl", n, m), S ** 3 * blv[n][sl])
            putb(("br", n, m), S ** 3 * brv[n][sl])
            putb(("bc", n, m), S * bcv[n][sl])
            putb(("bt", n, m), S * btv[n][sl])
    for m in range(2):
        sl = slice(m * 128, (m + 1) * 128)
        putb(("cb1", m), S * S * f32a(inputs["c_b1"])[sl])
        putb(("tb1", m), S * S * f32a(inputs["t_b1"])[sl])
        putb(("ab1", m), S * S * f32a(inputs["a_b1"])[sl])
    finb = np.zeros(128, np.float32)
    for i in range(N):
        finb[i * 10:i * 10 + 8] = f32a(inputs["a_b2"])[0]
        finb[i * 10 + 8] = f32a(inputs["c_b2"])[0]
        finb[i * 10 + 9] = f32a(inputs["t_b2"])[0]
    putb(("finb",), finb)

    zero_bias = all(
        not np.any(f32a(inputs[k]))
        for k in ("o_b0", "o_b1", "o_b2", "c_b0", "c_b1", "t_b0", "t_b1",
                  "a_b0", "a_b1")
    )

    ow0v = np.zeros((IN, N * H), bf)
    oW0 = f32a(inputs["o_W0"])
    for n in range(N):
        ow0v[:, n * H:(n + 1) * H] = (S * oW0[n]).astype(bf)

    xT = np.ascontiguousarray(f32a(inputs["x"]).T)
    common = {"wb": wbv.reshape(128, -1), "bias": biasv, "ow0": ow0v}
    in_maps = []
    for c in range(NCORES):
        m = dict(common)
        m["x"] = np.ascontiguousarray(xT[:, c * BC:(c + 1) * BC].astype(bf))
        in_maps.append(m)
    return in_maps, zero_bias


def run(inputs, trace=False, **kw):
    in_maps, zero_bias = _prep_inputs(inputs)
    key = ("nc", zero_bias)
    if key not in _CACHE:
        _CACHE[key] = _build(zero_bias)
    nc = _CACHE[key]
    res = run_bass_kernel_spmd(nc, in_maps, list(range(NCORES)), trace=trace, **kw)
    out = np.concatenate([res.results[c]["outT"].T for c in range(NCORES)], axis=0)
    return out.astype(np.float32), res


def kernel(**inputs) -> np.ndarray:
    out, _ = run(inputs, trace=False)
    return out


# revision 19
# speedup vs baseline: 1.0624x; 1.0624x over previous
"""BlockStackingSGN kernel for 8 Trainium2 NeuronCores.

Data-parallel over batch B=4096 (512 rows/core; batch in the free dim,
hidden on partitions). Key optimizations over a bf16 tiling:

- fp8e4m3 DoubleRow matmuls for every 256-deep contraction: one PE
  instruction contracts both 128-row k-tiles in the cycles of one,
  halving PE time.
- The linear object-encoder output layer (no relu) is folded on the host
  into its four downstream consumers (AonB-left/right, clear, ontable
  first layers), deleting that layer's matmuls and evacuations.
- Power-of-2 scaling (weights x16) keeps fp8 weights out of the
  subnormal range; scales flow through relu/add transparently and are
  absorbed for free by activation-engine scale or a tensor_scalar
  multiply, so every PSUM evacuation is a single instruction.
- Early phases run two 256-wide layers per 4-bank PSUM tile so one
  evacuation instruction drains four matmul accumulations (GpSimd
  cannot read PSUM, so evacuations are split across Scalar+Vector only;
  GpSimd handles the SBUF-side pair adds and relu casts).
- All 80 output heads (AonB pairs / clear / ontable) accumulate into one
  PSUM bank via one-hot fp8 stationaries sliced from a sliding window;
  a single batched Sigmoid finishes the kernel.
"""

import sys

import numpy as np

sys.path.insert(0, "/opt/trn_rl_repo")

import concourse.bacc as bacc
import concourse.mybir as mybir
import concourse.tile as tile
from concourse.bass_utils import run_bass_kernel_spmd

dt = mybir.dt
AF = mybir.ActivationFunctionType
ALU = mybir.AluOpType
PM = mybir.MatmulPerfMode

N = 8
H = 256
B = 4096
IN = 3 * N
NCORES = 8
BC = B // NCORES          # 512 batch rows per core
W = BC
R = N * (N + 2)           # 80 output rows
S = 16.0                  # weight scale 2^4

F32 = dt.float32
BF16 = dt.bfloat16
FP8 = dt.float8e4

_CACHE = {}


def _wb_layout():
    """fp8 weight tile entries of [128, 2, 256] (512 cols each), ordered by
    first use (doubles as DMA arrival order)."""
    keys = []
    for n in range(N):
        keys.append(("oW1", n))
    for n in range(N):
        keys.append(("Wl", n))
        keys.append(("Wr", n))
    for n in range(N):
        keys.append(("Wc", n))
        keys.append(("Wt", n))
    keys += [("cW1",), ("tW1",), ("w2c",), ("w2t",), ("ident",), ("aW1",),
             ("w2a",)]
    return {k: i for i, k in enumerate(keys)}, len(keys)


WB_ENT, WB_N = _wb_layout()
WB_COLS = WB_N * 512


def _bias_layout():
    keys = []
    for n in range(N):
        for nm in ("b0", "b1", "bl", "br", "bc", "bt"):
            for m in range(2):
                keys.append((nm, n, m))
    for nm in ("cb1", "tb1", "ab1", "zz"):
        for m in range(2):
            keys.append((nm, m))
    keys.append(("finb",))
    return {k: i for i, k in enumerate(keys)}, len(keys)


BIAS_OFF, BIAS_COLS = _bias_layout()

N_DMA_CHUNKS = 8


def _build(zero_bias):
    nc = bacc.Bacc("TRN2", target_bir_lowering=False, debug=False, num_devices=NCORES)

    d_x = nc.dram_tensor("x", [IN, BC], BF16, kind="ExternalInput")
    d_ow0 = nc.dram_tensor("ow0", [IN, N * H], BF16, kind="ExternalInput")
    d_wb = nc.dram_tensor("wb", [128, WB_COLS], FP8, kind="ExternalInput")
    d_bias = nc.dram_tensor("bias", [128, BIAS_COLS], F32, kind="ExternalInput")
    d_out = nc.dram_tensor("outT", [R, BC], F32, kind="ExternalOutput")

    K2 = 2 * W   # 1024: one 256-wide activation (2 k-tiles x 512 batch)

    with tile.TileContext(nc) as tc:
        with (
            tc.tile_pool(name="w", bufs=1) as wp,
            tc.tile_pool(name="act", bufs=1) as acp,
            tc.tile_pool(name="wk", bufs=8) as wk,
            tc.tile_pool(name="ph2", bufs=10) as php,
        ):
            xw = wp.tile([IN, BC], BF16, tag="xw")
            nc.sync.dma_start(xw[:], d_x[:])
            xT = xw[:]
            ow0 = wp.tile([IN, N * H], BF16, tag="ow0")
            nc.scalar.dma_start(ow0[:, :2 * H], d_ow0[:, :2 * H])
            nc.gpsimd.dma_start(ow0[:, 2 * H:], d_ow0[:, 2 * H:])
            bias = wp.tile([128, BIAS_COLS], F32, tag="bias")
            nc.gpsimd.dma_start(bias[:], d_bias[:])

            wb = wp.tile([128, WB_N, 2, 256], FP8, tag="wb")
            chunk = (WB_N + N_DMA_CHUNKS - 1) // N_DMA_CHUNKS
            for c in range(N_DMA_CHUNKS):
                eng = nc.gpsimd if c % 2 == 0 else nc.sync
                lo, hi = c * chunk, min((c + 1) * chunk, WB_N)
                if lo < hi:
                    eng.dma_start(wb[:, lo:hi], d_wb[:, lo * 512:hi * 512])

            def wsl(key, m):
                return wb[:, WB_ENT[key], :, m * 128:(m + 1) * 128]

            def w2sl(key, r):
                return wb[:, WB_ENT[key], :, 128 - r:256 - r]

            def bcol(key):
                return bias[:, BIAS_OFF[key]:BIAS_OFF[key] + 1]

            def engine(e):
                return {"A": nc.scalar, "D": nc.vector, "P": nc.gpsimd}[e]

            def asdr(ap2d):
                """view a [128, 1024] activation slice as DR rhs [128, 2, 512]"""
                return ap2d.rearrange("p (k w) -> p k w", k=2)

            def evac(e, out2, ps2, bkeys, relu, scale):
                """out = func(scale * psum + scale*bias). One instruction when
                biases are zero, else one per [128, W] column block."""
                if zero_bias:
                    if e == "A":
                        func = AF.Relu if relu else AF.Identity
                        nc.scalar.activation(out2, ps2, func, scale=scale)
                    elif relu:
                        # (mult scale, max 0): measured faster than plain max
                        engine(e).tensor_scalar(out2, ps2, scale, 0.0,
                                                ALU.mult, ALU.max)
                    else:
                        engine(e).tensor_scalar(out2, ps2, scale, None, ALU.mult)
                else:
                    nsub = out2.shape[-1] // W if len(out2.shape) == 2 else 2
                    for m in range(nsub):
                        o = out2[:, m * W:(m + 1) * W]
                        p = ps2[:, m * W:(m + 1) * W]
                        b = bcol(bkeys[m])
                        if e == "A" or (relu and scale != 1.0):
                            func = AF.Relu if relu else AF.Identity
                            nc.scalar.activation(o, p, func, bias=b, scale=scale)
                        elif relu:
                            engine(e).tensor_scalar(o, p, b, 0.0, ALU.add, ALU.max)
                        else:
                            engine(e).tensor_scalar(o, p, b, scale,
                                                    ALU.add, ALU.mult)

            cts = {}

            def pick(seq, key):
                c = cts.setdefault(key, [0])
                e = seq[c[0] % len(seq)]
                c[0] += 1
                return e

            SEQ_ERA = "ADAD"      # era-A 4-bank evacs
            SEQ_Y1 = "AD"
            SEQ_YE = "DADA"       # pair y evacs

            # ======= single psum pool for the whole kernel (no phase
            # barrier: the 3-tile rotation pipelines through boundaries) ====
            h0 = acp.tile([128, N * K2], FP8, tag="h0")
            h1 = acp.tile([128, N * K2], FP8, tag="h1")
            alr = acp.tile([128, 2 * N * K2], FP8, tag="alr")

            def blk(t, n, q=1):
                return t[:, n * K2:(n + q) * K2]

            import bass_rust as _br

            def pair_rhs(i, j, m):
                """[128, 2, 512] AP: dim1 hops from al_i[m] to ar_j[m]."""
                sl = alr[:, 2 * i * K2 + m * W: 2 * i * K2 + (m + 1) * W].copy()
                step = (2 * j + 1 - 2 * i) * K2
                sl.ap = _br.VecI64Pair([[2 * N * K2, 128], [step, 2], [1, W]])
                return sl

            # ================= era B: pair loop + preds =================
            with (
                tc.tile_pool(name="py", bufs=3, space="PSUM") as py,
                tc.tile_pool(name="pf", bufs=1, space="PSUM") as pf,
            ):
                pp = py
                fin = pf.tile([128, BC], F32, tag="fin")

                # ---- L0 (bf16, contraction 24) ----
                for n in range(N):
                    pst = py.tile([128, K2], F32, tag="py", name=f"psA{n}")
                    for m in range(2):
                        o = n * H + m * 128
                        nc.tensor.matmul(pst[:, m * W:(m + 1) * W],
                                         ow0[:, o:o + 128], xT, start=True,
                                         stop=True, skip_group_check=True)
                    evac(pick(SEQ_ERA, "h0"), blk(h0, n), pst[:],
                         [("b0", n, m) for m in range(2)], True, 1.0)
                # ---- L1 (DR) ----
                for n in range(N):
                    pst = py.tile([128, K2], F32, tag="py", name=f"psB{n}")
                    for m in range(2):
                        nc.tensor.matmul(pst[:, m * W:(m + 1) * W],
                                         wsl(("oW1", n), m), asdr(blk(h0, n)),
                                         start=True, stop=True,
                                         perf_mode=PM.DoubleRow,
                                         skip_group_check=True)
                    evac(pick(SEQ_ERA, "h1"), blk(h1, n), pst[:],
                         [("b1", n, m) for m in range(2)], True, 1.0)
                # ---- al / ar (DR, enc folded; fp8 out at 2^4, interleaved) --
                for n in range(N):
                    for q, (key, bk) in enumerate((("Wl", "bl"), ("Wr", "br"))):
                        pst = py.tile([128, K2], F32, tag="py",
                                      name=f"psC{n}{key}")
                        for m in range(2):
                            nc.tensor.matmul(pst[:, m * W:(m + 1) * W],
                                             wsl((key, n), m), asdr(blk(h1, n)),
                                             start=True, stop=True,
                                             perf_mode=PM.DoubleRow,
                                             skip_group_check=True)
                        evac(pick(SEQ_ERA, "al"),
                             alr[:, (2 * n + q) * K2:(2 * n + q + 1) * K2],
                             pst[:], [(bk, n, m) for m in range(2)],
                             False, 1.0 / (S * S))
                n_fin = N * N + 2 * N
                fin_ct = [0]

                def fin_mm(w2key, r, rhs2):
                    first = fin_ct[0] == 0
                    fin_ct[0] += 1
                    last = fin_ct[0] == n_fin
                    nc.tensor.matmul(fin[:], w2sl(w2key, r), asdr(rhs2),
                                     start=first, stop=last,
                                     perf_mode=PM.DoubleRow)

                def dr2(ps2, key, rhs2):
                    for m in range(2):
                        nc.tensor.matmul(ps2[:, m * W:(m + 1) * W], wsl(key, m),
                                         asdr(rhs2), start=True, stop=True,
                                         perf_mode=PM.DoubleRow,
                                         skip_group_check=True)

                def pred_stages(n, w0k, w1k, b0k, b1k, w2k, r):
                    """three independently schedulable stages of one predicate"""
                    st = {}

                    def s1():
                        y0 = wk.tile([128, K2], FP8, tag="y0",
                                     name=f"y0_{n}_{w0k}")
                        pst = pp.tile([128, K2], F32, tag="py",
                                      name=f"psY0{n}{w0k}")
                        dr2(pst, (w0k, n), blk(h1, n))
                        evac("A", y0[:], pst[:],
                             [(b0k, n, m) for m in range(2)], True, 1.0 / (S * S))
                        st["y0"] = y0

                    def s2():
                        y1 = wk.tile([128, K2], FP8, tag="y1",
                                     name=f"y1_{n}_{w0k}")
                        pst2 = pp.tile([128, K2], F32, tag="py",
                                       name=f"psY1{n}{w0k}")
                        dr2(pst2, w1k, st["y0"][:])
                        evac(pick(SEQ_Y1, "y1"), y1[:], pst2[:],
                             [(b1k, m) for m in range(2)], True, 1.0)
                        st["y1"] = y1

                    def s3():
                        fin_mm(w2k, r, st["y1"][:])

                    return [s1, s2, s3]

                allst = []
                for n in range(N):
                    allst.append(pred_stages(n, "Wc", ("cW1",), "bc", "cb1",
                                             ("w2c",), n * 10 + 8))
                    allst.append(pred_stages(n, "Wt", ("tW1",), "bt", "tb1",
                                             ("w2t",), n * 10 + 9))
                # interleave stages of pred pairs: A1 B1 A2 B2 A3 B3
                stageq = []
                for pn in range(0, 16, 2):
                    A, Bq = allst[pn], allst[pn + 1]
                    stageq += [A[0], Bq[0], A[1], Bq[1], A[2], Bq[2]]

                # modulo-scheduled pair loop: ident leads, aW1 lags 1 pair,
                # fin lags 4 pairs; pred stages fill the gaps
                PAIRS = sorted(((i, j) for i in range(N)
                                for j in range(N)),
                               key=lambda p: (max(p), p))
                PHQ, YQ = {}, {}

                def emit_ident(k):
                    i, j = PAIRS[k]
                    pst1 = py.tile([128, K2], F32, tag="py", name=f"psH{i}{j}")
                    for m in range(2):
                        nc.tensor.matmul(pst1[:, m * W:(m + 1) * W],
                                         wsl(("ident",), 0), pair_rhs(i, j, m),
                                         start=True, stop=True,
                                         perf_mode=PM.DoubleRow,
                                         skip_group_check=True)
                    ph = php.tile([128, K2], FP8, tag="ph", name=f"ph{i}{j}")
                    evac(pick("ADAD", "ph"), ph[:], pst1[:],
                         [("zz", m) for m in range(2)], True, 1.0)
                    PHQ[k] = ph

                def emit_aw1(k):
                    i, j = PAIRS[k]
                    ph = PHQ.pop(k)
                    pst = py.tile([128, K2], F32, tag="py", name=f"psP{i}{j}")
                    for m in range(2):
                        nc.tensor.matmul(
                            pst[:, m * W:(m + 1) * W], wsl(("aW1",), m),
                            asdr(ph[:]), start=True, stop=True,
                            perf_mode=PM.DoubleRow, skip_group_check=True)
                    y = wk.tile([128, K2], FP8, tag="y", name=f"y_{i}{j}")
                    evac(pick(SEQ_YE, "ye"), y[:], pst[:],
                         [("ab1", m) for m in range(2)], True, 1.0)
                    YQ[k] = y

                def emit_fin(k):
                    i, j = PAIRS[k]
                    y = YQ.pop(k)
                    fin_mm(("w2a",), i * 10 + j, y[:])

                NP = len(PAIRS)
                for k in range(NP + 4):
                    if k < NP:
                        emit_ident(k)
                    if k % 4 != 0 and stageq:
                        stageq.pop(0)()
                    if 1 <= k <= NP:
                        emit_aw1(k - 1)
                    if k >= 4:
                        emit_fin(k - 4)
                while stageq:
                    stageq.pop(0)()
                assert fin_ct[0] == n_fin

                # batched sigmoid + store
                outT = wk.tile([128, BC], F32, tag="outT")
                nc.scalar.activation(outT[:], fin[:], AF.Sigmoid,
                                     bias=bcol(("finb",)), scale=1.0 / (S ** 3))
                nc.sync.dma_start(d_out[:], outT[:R, :])

    nc.compile()
    return nc


def _prep_inputs(inputs):
    import ml_dtypes

    bf = ml_dtypes.bfloat16
    f8 = ml_dtypes.float8_e4m3fn
    f32a = lambda a: np.asarray(a, dtype=np.float32)

    wbv = np.zeros((128, WB_N, 2, 256), f8)

    def put(key, Wmat):  # Wmat: [256, 256] fp32, already scaled
        e = WB_ENT[key]
        for k in range(2):
            wbv[:, e, k, :] = Wmat[k * 128:(k + 1) * 128].astype(f8)

    oW1 = f32a(inputs["o_W1"])
    oW2 = f32a(inputs["o_W2"])
    aW0 = f32a(inputs["a_W0"])
    cW0 = f32a(inputs["c_W0"])
    tW0 = f32a(inputs["t_W0"])
    for n in range(N):
        put(("oW1", n), S * oW1[n])
        put(("Wl", n), S * (oW2[n] @ aW0[:H]))
        put(("Wr", n), S * (oW2[n] @ aW0[H:]))
        put(("Wc", n), S * (oW2[n] @ cW0))
        put(("Wt", n), S * (oW2[n] @ tW0))
    eye = np.zeros((2 * 128, 256), np.float32)
    eye[:128, :128] = np.eye(128)
    eye[128:, :128] = np.eye(128)
    put(("ident",), eye)
    put(("cW1",), S * f32a(inputs["c_W1"]))
    put(("tW1",), S * f32a(inputs["t_W1"]))
    put(("aW1",), S * f32a(inputs["a_W1"]))
    for key, src in ((("w2c",), "c_W2"), (("w2t",), "t_W2"), (("w2a",), "a_W2")):
        w2 = S * f32a(inputs[src])[:, 0]
        e = WB_ENT[key]
        for k in range(2):
            wbv[:, e, k, 128] = w2[k * 128:(k + 1) * 128].astype(f8)

    biasv = np.zeros((128, BIAS_COLS), np.float32)

    def putb(key, vec):
        biasv[:, BIAS_OFF[key]] = vec

    ob2 = f32a(inputs["o_b2"])
    blv = ob2 @ aW0[:H] + f32a(inputs["a_b0"])[None, :]
    brv = ob2 @ aW0[H:]
    bcv = ob2 @ cW0 + f32a(inputs["c_b0"])[None, :]
    btv = ob2 @ tW0 + f32a(inputs["t_b0"])[None, :]
    for n in range(N):
        for m in range(2):
            sl = slice(m * 128, (m + 1) * 128)
            putb(("b0", n, m), S * f32a(inputs["o_b0"])[n][sl])
            putb(("b1", n, m), S * S * f32a(inputs["o_b1"])[n][sl])
            putb(("bl", n, m), S ** 3 * blv[n][sl])
            putb(("br", n, m), S ** 3 * brv[n][sl])
            putb(("bc", n, m), S * bcv[n][sl])
            putb(("bt", n, m), S * btv[n][sl])
    for m in range(2):
        sl = slice(m * 128, (m + 1) * 128)
        putb(("cb1", m), S * S * f32a(inputs["c_b1"])[sl])
        putb(("tb1", m), S * S * f32a(inputs["t_b1"])[sl])
        putb(("ab1", m), S * S * f32a(inputs["a_b1"])[sl])
    finb = np.zeros(128, np.float32)
    for i in range(N):
        finb[i * 10:i * 10 + 8] = f32a(inputs["a_b2"])[0]
        finb[i * 10 + 8] = f32a(inputs["c_b2"])[0]
        finb[i * 10 + 9] = f32a(inputs["t_b2"])[0]
    putb(("finb",), finb)

    zero_bias = all(
        not np.any(f32a(inputs[k]))
        for k in ("o_b0", "o_b1", "o_b2", "c_b0", "c_b1", "t_b0", "t_b1",
                  "a_b0", "a_b1")
    )

    ow0v = np.zeros((IN, N * H), bf)
    oW0 = f32a(inputs["o_W0"])
    for n in range(N):
        ow0v[:, n * H:(n + 1) * H] = (S * oW0[n]).astype(bf)

    xT = np.ascontiguousarray(f32a(inputs["x"]).T)
    common = {"wb": wbv.reshape(128, -1), "bias": biasv, "ow0": ow0v}
    in_maps = []
    for c in range(NCORES):
        m = dict(common)
        m["x"] = np.ascontiguousarray(xT[:, c * BC:(c + 1) * BC].astype(bf))
        in_maps.append(m)
    return in_maps, zero_bias


def run(inputs, trace=False, **kw):
    in_maps, zero_bias = _prep_inputs(inputs)
    key = ("nc", zero_bias)
    if key not in _CACHE:
        _CACHE[key] = _build(zero_bias)
    nc = _CACHE[key]
    res = run_bass_kernel_spmd(nc, in_maps, list(range(NCORES)), trace=trace, **kw)
    out = np.concatenate([res.results[c]["outT"].T for c in range(NCORES)], axis=0)
    return out.astype(np.float32), res


def kernel(**inputs) -> np.ndarray:
    out, _ = run(inputs, trace=False)
    return out
